# revision 4
# baseline (speedup 1.0000x reference)
"""AMICO ADMM solver on 8 Trainium2 NeuronCores.

Problem: X = argmin ||Y^T - A x||^2 + lam*||x||_1 s.t. x >= 0, solved with
max_iter ADMM steps (rho=1, lam=0.1) exactly as in the reference scan.

Algebraic reduction (tracking only v = x + u):
    v_1 = G                      with G  = Minv @ A^T @ Y^T
    for i = 2..N:
        w   = |v - t|            (t = lam/rho)
        S   = min(v, t) + Gb     (Gb = G - t * Minv @ 1)
        v'  = Minv @ w + S
    output x_N = Minv @ w_{N-1} + Gb

since z = relu(v - t), u' = v - z = min(v, t), and z - u' = |v - t| - t.
The constant -t*Minv@1 and the A^T Y^T term are folded into a single
"augmented" matmul: Gb = Ht_aug^T @ Yt_aug where Ht_aug carries A@Minv plus a
bias row (-t * rowsum(Minv)) and Yt_aug carries Y^T plus a row of ones.

Sharding: data-parallel over voxels (B=4096 -> 512 per core); A-derived
matrices (Minv, Ht_aug) replicated; no cross-core communication.

Device work per core per iteration: 16 fp32r matmuls [128x128 @ 128x512]
(PE, m-outer emission so each PSUM chunk completes early and feeds the
next iteration's weights-chain with minimal stall), one Abs activation per
chunk (ACT), one fused scalar_tensor_tensor V-op per chunk (DVE), and the
S-ops split 2/2 between DVE (fused) and GPSIMD (min + add pair).
"""

import numpy as np

B_VOX = 4096
M_MEAS = 256
K_ATOMS = 512
P = 128
N_CORES = 8
BS = B_VOX // N_CORES  # 512 voxels per core
KB = K_ATOMS // P  # 4 chunks of the contraction/output dim
LAM = 0.1
RHO = 1.0
THR = LAM / RHO

_NC_CACHE = {}


def _build(niter):
    import concourse.mybir as mybir
    import concourse.tile as tile
    from concourse import bacc

    f32 = mybir.dt.float32
    f32r = mybir.dt.float32r
    Alu = mybir.AluOpType
    Act = mybir.ActivationFunctionType

    nc = bacc.Bacc(None, target_bir_lowering=False)
    ht = nc.declare_dram_parameter("Ht", [3 * P, K_ATOMS], f32r, isOutput=False)
    yt = nc.declare_dram_parameter("Yt", [3 * P, BS], f32r, isOutput=False)
    mi = nc.declare_dram_parameter("Mi", [K_ATOMS, K_ATOMS], f32r, isOutput=False)
    rs = nc.declare_dram_parameter("rs", [P, KB], f32, isOutput=False)
    out = nc.declare_dram_parameter("out", [K_ATOMS, BS], f32, isOutput=True)

    with tile.TileContext(nc) as tc:
        with (
            tc.tile_pool(name="const", bufs=1) as cpool,
            tc.tile_pool(name="v", bufs=8) as vpool,
            tc.tile_pool(name="w", bufs=12) as wpool,
            tc.tile_pool(name="s", bufs=8) as spool,
            tc.tile_pool(name="g", bufs=4) as gpool,
            tc.tile_pool(name="o", bufs=4) as opool,
            tc.tile_pool(name="psum", bufs=8, space="PSUM") as ppool,
        ):
            nb = cpool.tile([P, 1], f32)
            nc.vector.memset(nb[:], -THR)
            ht_sb = cpool.tile([P, 3, K_ATOMS], f32r)
            nc.sync.dma_start(ht_sb[:], ht.rearrange("(kb p) a -> p kb a", p=P))
            yt_sb = cpool.tile([P, 3, BS], f32r)
            nc.sync.dma_start(yt_sb[:], yt.rearrange("(kb p) b -> p kb b", p=P))
            mi_sb = cpool.tile([P, KB, K_ATOMS], f32r)
            nc.sync.dma_start(mi_sb[:], mi.rearrange("(kb p) m -> p kb m", p=P))
            rs_sb = cpool.tile([P, KB], f32)
            nc.sync.dma_start(rs_sb[:], rs[:])
            gb_sb = cpool.tile([P, KB, BS], f32)

            outr = out.rearrange("(mb p) n -> mb p n", p=P)

            w_cur = [None] * KB
            s_cur = [None] * KB

            # ---- iteration 1: Gb = Ht_aug^T @ Yt_aug (m-outer blocks) ----
            pgs = [
                ppool.tile([P, BS], f32, tag="pp", name=f"pg{m}") for m in range(KB)
            ]
            for m in range(KB):
                for kb in range(3):
                    nc.tensor.matmul(
                        pgs[m][:],
                        lhsT=ht_sb[:, kb, m * P : (m + 1) * P],
                        rhs=yt_sb[:, kb, :],
                        start=(kb == 0),
                        stop=(kb == 2),
                    )
                if niter == 1:
                    xm = opool.tile([P, BS], f32, tag="x", name=f"x1{m}")
                    nc.vector.tensor_scalar_add(xm[:], pgs[m][:], rs_sb[:, m : m + 1])
                    nc.sync.dma_start(outr[m], xm[:])
                else:
                    # v_1 = G = Gb + t*rowsum(Minv)
                    vm = vpool.tile([P, BS], f32, tag="v", name=f"v1{m}")
                    nc.vector.tensor_scalar_add(vm[:], pgs[m][:], rs_sb[:, m : m + 1])
                    wm = wpool.tile([P, BS], f32r, tag="w", name=f"w1{m}")
                    nc.scalar.activation(wm[:], vm[:], Act.Abs, bias=nb[:, 0:1])
                    # Gb to SBUF (needed every iteration)
                    nc.scalar.activation(gb_sb[:, m, :], pgs[m][:], Act.Copy)
                    sm = spool.tile([P, BS], f32, tag="s", name=f"s1{m}")
                    nc.vector.scalar_tensor_tensor(
                        sm[:], vm[:], THR, gb_sb[:, m, :], Alu.min, Alu.add
                    )
                    w_cur[m], s_cur[m] = wm, sm

            # ---- iterations 2..niter ----
            for it in range(2, niter + 1):
                last = it == niter
                pps = [
                    ppool.tile([P, BS], f32, tag="pp", name=f"pp{it}_{m}")
                    for m in range(KB)
                ]
                vs = [None] * KB
                neww = [None] * KB
                news = [None] * KB
                for m in range(KB):
                    # m-outer: finish this output chunk's accumulation early
                    for kb in range(KB):
                        nc.tensor.matmul(
                            pps[m][:],
                            lhsT=mi_sb[:, kb, m * P : (m + 1) * P],
                            rhs=w_cur[kb][:],
                            start=(kb == 0),
                            stop=(kb == KB - 1),
                        )
                    if last:
                        xm = opool.tile([P, BS], f32, tag="x", name=f"x{m}")
                        nc.vector.scalar_tensor_tensor(
                            xm[:], pps[m][:], 0.0, gb_sb[:, m, :], Alu.bypass, Alu.add
                        )
                        nc.sync.dma_start(outr[m], xm[:])
                    else:
                        # V-op: v = psum + S_prev  (critical chain, keep DVE free)
                        vm = vpool.tile([P, BS], f32, tag="v", name=f"v{it}_{m}")
                        nc.vector.scalar_tensor_tensor(
                            vm[:], pps[m][:], 0.0, s_cur[m][:], Alu.bypass, Alu.add
                        )
                        vs[m] = vm
                        wm = wpool.tile([P, BS], f32r, tag="w", name=f"w{it}_{m}")
                        nc.scalar.activation(wm[:], vm[:], Act.Abs, bias=nb[:, 0:1])
                        neww[m] = wm
                if not last:
                    # S-ops off the critical chain: 2 on DVE (fused), 2 on GPSIMD
                    for m in range(KB):
                        sm = spool.tile([P, BS], f32, tag="s", name=f"s{it}_{m}")
                        if m < 2:
                            nc.vector.scalar_tensor_tensor(
                                sm[:], vs[m][:], THR, gb_sb[:, m, :], Alu.min, Alu.add
                            )
                        else:
                            gm = gpool.tile([P, BS], f32, tag="gmin", name=f"gm{it}_{m}")
                            nc.gpsimd.tensor_scalar_min(gm[:], vs[m][:], THR)
                            nc.gpsimd.tensor_add(sm[:], gm[:], gb_sb[:, m, :])
                        news[m] = sm
                    w_cur, s_cur = neww, news

    nc.finalize()
    return nc


def _get_nc(niter):
    if niter not in _NC_CACHE:
        _NC_CACHE[niter] = _build(niter)
    return _NC_CACHE[niter]


def _prep_in_maps(Y, A):
    """Host precompute of the A-derived (voxel-independent) factor matrices,
    in float64: the inverse replaces the reference's Cholesky solve. Shards Y
    over voxels (transposed, with the augmented ones-row appended)."""
    A64 = A.astype(np.float64)
    LHS = A64.T @ A64 + RHO * np.eye(K_ATOMS)
    Minv = np.linalg.inv(LHS)
    Minv = (Minv + Minv.T) / 2
    Hm = A64 @ Minv  # [M, K]
    rsum = Minv.sum(axis=1)

    Ht = np.zeros((3 * P, K_ATOMS), np.float32)
    Ht[:M_MEAS] = Hm.astype(np.float32)
    Ht[M_MEAS] = (-THR * rsum).astype(np.float32)
    Mi = Minv.astype(np.float32)
    rs = np.ascontiguousarray((THR * rsum).astype(np.float32).reshape(KB, P).T)

    in_maps = []
    for c in range(N_CORES):
        Yt = np.zeros((3 * P, BS), np.float32)
        Yt[:M_MEAS] = Y[c * BS : (c + 1) * BS, :].T
        Yt[M_MEAS] = 1.0
        in_maps.append(
            {"Yt": np.ascontiguousarray(Yt), "Ht": Ht, "Mi": Mi, "rs": rs}
        )
    return in_maps


def kernel(Y, A, max_iter):
    from concourse.bass_utils import run_bass_kernel_spmd

    Y = np.ascontiguousarray(np.asarray(Y, dtype=np.float32))
    A = np.ascontiguousarray(np.asarray(A, dtype=np.float32))
    niter = int(max_iter)
    assert Y.shape == (B_VOX, M_MEAS) and A.shape == (M_MEAS, K_ATOMS)
    assert niter >= 1

    in_maps = _prep_in_maps(Y, A)
    nc = _get_nc(niter)
    res = run_bass_kernel_spmd(nc, in_maps, core_ids=list(range(N_CORES)))

    outp = np.empty((B_VOX, K_ATOMS), np.float32)
    for c in range(N_CORES):
        outp[c * BS : (c + 1) * BS] = res.results[c]["out"].T
    return outp


# revision 6
# speedup vs baseline: 3.1278x; 3.1278x over previous
"""AMICO ADMM solver on 8 Trainium2 NeuronCores.

Problem: X = argmin ||Y^T - A x||^2 + lam*||x||_1 s.t. x >= 0, solved with
max_iter ADMM steps (rho=1, lam=0.1) exactly as in the reference scan.

Algebraic reduction (tracking only v = x + u):
    v_1 = G                      with G  = Minv @ A^T @ Y^T
    for i = 2..N:
        w   = |v - t|            (t = lam/rho)
        S   = min(v, t) + Gb     (Gb = G - t * Minv @ 1)
        v'  = Minv @ w + S
    output x_N = Minv @ w_{N-1} + Gb

since z = relu(v - t), u' = v - z = min(v, t), and z - u' = |v - t| - t.
The constant -t*Minv@1 and the A^T Y^T term are folded into a single
"augmented" matmul: Gb = Ht_aug^T @ Yt_aug where Ht_aug carries A@Minv plus a
bias row (-t * rowsum(Minv)) and Yt_aug carries Y^T plus a row of ones.

Sharding: data-parallel over voxels (B=4096 -> 512 per core); A-derived
matrices (Minv, Ht_aug) replicated; no cross-core communication.

Implementation notes:
 - Minv is stored as fp16 weights (11-bit mantissa; enables the compiler's
   fast-weight-load path so LDWEIGHTS hides under the previous matmul);
   the moving operand w is fp16 as well (mixed 16/32-bit matmul operands are unsupported). Measured end-to-end error vs
   the float32 cho_solve reference is ~3e-3.
 - Output chunks 0,1 use a DVE V-op (v = psum + S); chunks 2,3 instead
   accumulate I @ S_comb into the PSUM group via an identity matmul, so v
   materializes directly in PSUM and the Abs activation reads it from there,
   shortening the cross-iteration dependency chain. This splits per-iteration
   work evenly: PE 18 matmuls, DVE 6 fused ops, ACT 4 activations.
"""

import numpy as np

B_VOX = 4096
M_MEAS = 256
K_ATOMS = 512
P = 128
N_CORES = 8
BS = B_VOX // N_CORES  # 512 voxels per core
KB = K_ATOMS // P  # 4 chunks of the contraction/output dim
N_IDENT = 2  # chunks KB-N_IDENT..KB-1 use the identity-accumulate scheme
LAM = 0.1
RHO = 1.0
THR = LAM / RHO

_NC_CACHE = {}


def _build(niter):
    import concourse.mybir as mybir
    import concourse.tile as tile
    from concourse import bacc

    f32 = mybir.dt.float32
    f32r = mybir.dt.float32r
    f16 = mybir.dt.float16
    Alu = mybir.AluOpType
    Act = mybir.ActivationFunctionType

    nc = bacc.Bacc(None, target_bir_lowering=False)
    ht = nc.declare_dram_parameter("Ht", [3 * P, K_ATOMS], f32r, isOutput=False)
    yt = nc.declare_dram_parameter("Yt", [3 * P, BS], f32r, isOutput=False)
    mi = nc.declare_dram_parameter("Mi", [K_ATOMS, K_ATOMS], f16, isOutput=False)
    rs = nc.declare_dram_parameter("rs", [P, KB], f32, isOutput=False)
    ident = nc.declare_dram_parameter("Id", [P, P], f16, isOutput=False)
    out = nc.declare_dram_parameter("out", [K_ATOMS, BS], f32, isOutput=True)

    ID_CHUNK = KB - N_IDENT  # m >= ID_CHUNK -> identity-accumulate scheme

    with tile.TileContext(nc) as tc:
        with (
            tc.tile_pool(name="const", bufs=1) as cpool,
            tc.tile_pool(name="v", bufs=8) as vpool,
            tc.tile_pool(name="w", bufs=12) as wpool,
            tc.tile_pool(name="s", bufs=8) as spool,
            tc.tile_pool(name="o", bufs=4) as opool,
            tc.tile_pool(name="psum", bufs=8, space="PSUM") as ppool,
        ):
            nb = cpool.tile([P, 1], f32)
            nc.vector.memset(nb[:], -THR)
            ht_sb = cpool.tile([P, 3, K_ATOMS], f32r)
            nc.sync.dma_start(ht_sb[:], ht.rearrange("(kb p) a -> p kb a", p=P))
            yt_sb = cpool.tile([P, 3, BS], f32r)
            nc.sync.dma_start(yt_sb[:], yt.rearrange("(kb p) b -> p kb b", p=P))
            mi_sb = cpool.tile([P, KB, K_ATOMS], f16)
            nc.sync.dma_start(mi_sb[:], mi.rearrange("(kb p) m -> p kb m", p=P))
            rs_sb = cpool.tile([P, KB], f32)
            nc.sync.dma_start(rs_sb[:], rs[:])
            id_sb = cpool.tile([P, P], f16)
            nc.sync.dma_start(id_sb[:], ident[:])
            gb_sb = cpool.tile([P, KB, BS], f32)

            outr = out.rearrange("(mb p) n -> mb p n", p=P)

            w_cur = [None] * KB
            s_cur = [None] * KB  # classic chunks: S (f32); ident chunks: S_comb (f32r)

            # ---- iteration 1: Gb = Ht_aug^T @ Yt_aug (m-outer blocks) ----
            pgs = [
                ppool.tile([P, BS], f32, tag="pp", name=f"pg{m}") for m in range(KB)
            ]
            for m in range(KB):
                for kb in range(3):
                    nc.tensor.matmul(
                        pgs[m][:],
                        lhsT=ht_sb[:, kb, m * P : (m + 1) * P],
                        rhs=yt_sb[:, kb, :],
                        start=(kb == 0),
                        stop=(kb == 2),
                    )
                if niter == 1:
                    xm = opool.tile([P, BS], f32, tag="x", name=f"x1{m}")
                    nc.vector.tensor_scalar_add(xm[:], pgs[m][:], rs_sb[:, m : m + 1])
                    nc.sync.dma_start(outr[m], xm[:])
                else:
                    # v_1 = G = Gb + t*rowsum(Minv)
                    vm = vpool.tile([P, BS], f32, tag="v", name=f"v1{m}")
                    nc.vector.tensor_scalar_add(vm[:], pgs[m][:], rs_sb[:, m : m + 1])
                    wm = wpool.tile([P, BS], f16, tag="w", name=f"w1{m}")
                    nc.scalar.activation(wm[:], vm[:], Act.Abs, bias=nb[:, 0:1])
                    # Gb to SBUF (needed every iteration)
                    nc.scalar.activation(gb_sb[:, m, :], pgs[m][:], Act.Copy)
                    sdt = f32 if m < ID_CHUNK else f16
                    sm = spool.tile([P, BS], sdt, tag=f"s{m}", name=f"s1{m}")
                    nc.vector.scalar_tensor_tensor(
                        sm[:], vm[:], THR, gb_sb[:, m, :], Alu.min, Alu.add
                    )
                    w_cur[m], s_cur[m] = wm, sm

            # ---- iterations 2..niter ----
            for it in range(2, niter + 1):
                last = it == niter
                pps = [
                    ppool.tile([P, BS], f32, tag="pp", name=f"pp{it}_{m}")
                    for m in range(KB)
                ]
                vs = [None] * KB
                neww = [None] * KB
                news = [None] * KB
                for m in range(KB):
                    use_ident = (m >= ID_CHUNK) and not last
                    if use_ident:
                        # v' accumulates directly in PSUM: I @ S_comb + Minv @ w
                        nc.tensor.matmul(
                            pps[m][:],
                            lhsT=id_sb[:],
                            rhs=s_cur[m][:],
                            start=True,
                            stop=False,
                        )
                    for kb in range(KB):
                        nc.tensor.matmul(
                            pps[m][:],
                            lhsT=mi_sb[:, kb, m * P : (m + 1) * P],
                            rhs=w_cur[kb][:],
                            start=(kb == 0) and not use_ident,
                            stop=(kb == KB - 1),
                        )
                    if last:
                        xm = opool.tile([P, BS], f32, tag="x", name=f"x{m}")
                        nc.vector.scalar_tensor_tensor(
                            xm[:], pps[m][:], 0.0, gb_sb[:, m, :], Alu.bypass, Alu.add
                        )
                        nc.sync.dma_start(outr[m], xm[:])
                    elif m < ID_CHUNK:
                        # V-op: v = psum + S_prev (critical chain)
                        vm = vpool.tile([P, BS], f32, tag="v", name=f"v{it}_{m}")
                        nc.vector.scalar_tensor_tensor(
                            vm[:], pps[m][:], 0.0, s_cur[m][:], Alu.bypass, Alu.add
                        )
                        vs[m] = vm
                        wm = wpool.tile([P, BS], f16, tag="w", name=f"w{it}_{m}")
                        nc.scalar.activation(wm[:], vm[:], Act.Abs, bias=nb[:, 0:1])
                        neww[m] = wm
                    else:
                        # v lives in PSUM; ACT reads it directly
                        wm = wpool.tile([P, BS], f16, tag="w", name=f"w{it}_{m}")
                        nc.scalar.activation(wm[:], pps[m][:], Act.Abs, bias=nb[:, 0:1])
                        neww[m] = wm
                        sm = spool.tile([P, BS], f16, tag=f"s{m}", name=f"s{it}_{m}")
                        nc.vector.scalar_tensor_tensor(
                            sm[:], pps[m][:], THR, gb_sb[:, m, :], Alu.min, Alu.add
                        )
                        news[m] = sm
                if not last:
                    for m in range(ID_CHUNK):
                        sm = spool.tile([P, BS], f32, tag=f"s{m}", name=f"s{it}_{m}")
                        nc.vector.scalar_tensor_tensor(
                            sm[:], vs[m][:], THR, gb_sb[:, m, :], Alu.min, Alu.add
                        )
                        news[m] = sm
                    w_cur, s_cur = neww, news

    nc.finalize()
    return nc


def _get_nc(niter):
    if niter not in _NC_CACHE:
        _NC_CACHE[niter] = _build(niter)
    return _NC_CACHE[niter]


def _prep_in_maps(Y, A):
    """Host precompute of the A-derived (voxel-independent) factor matrices,
    in float64: the inverse replaces the reference's Cholesky solve. Shards Y
    over voxels (transposed, with the augmented ones-row appended)."""
    A64 = A.astype(np.float64)
    LHS = A64.T @ A64 + RHO * np.eye(K_ATOMS)
    Minv = np.linalg.inv(LHS)
    Minv = (Minv + Minv.T) / 2
    Hm = A64 @ Minv  # [M, K]
    rsum = Minv.sum(axis=1)

    Ht = np.zeros((3 * P, K_ATOMS), np.float32)
    Ht[:M_MEAS] = Hm.astype(np.float32)
    Ht[M_MEAS] = (-THR * rsum).astype(np.float32)
    Mi = Minv.astype(np.float16)
    rs = np.ascontiguousarray((THR * rsum).astype(np.float32).reshape(KB, P).T)
    Id = np.eye(P, dtype=np.float16)

    in_maps = []
    for c in range(N_CORES):
        Yt = np.zeros((3 * P, BS), np.float32)
        Yt[:M_MEAS] = Y[c * BS : (c + 1) * BS, :].T
        Yt[M_MEAS] = 1.0
        in_maps.append(
            {"Yt": np.ascontiguousarray(Yt), "Ht": Ht, "Mi": Mi, "rs": rs, "Id": Id}
        )
    return in_maps


def kernel(Y, A, max_iter):
    from concourse.bass_utils import run_bass_kernel_spmd

    Y = np.ascontiguousarray(np.asarray(Y, dtype=np.float32))
    A = np.ascontiguousarray(np.asarray(A, dtype=np.float32))
    niter = int(max_iter)
    assert Y.shape == (B_VOX, M_MEAS) and A.shape == (M_MEAS, K_ATOMS)
    assert niter >= 1

    in_maps = _prep_in_maps(Y, A)
    nc = _get_nc(niter)
    res = run_bass_kernel_spmd(nc, in_maps, core_ids=list(range(N_CORES)))

    outp = np.empty((B_VOX, K_ATOMS), np.float32)
    for c in range(N_CORES):
        outp[c * BS : (c + 1) * BS] = res.results[c]["out"].T
    return outp
